# revision 43
# baseline (speedup 1.0000x reference)
"""DynamicGCN edge-MLP message passing kernel for 8x trn2 NeuronCores.

Shapes (hardcoded): x [2, 512, 256] f32, base_adj [2, 512, 512] f32,
W1 [512, 256], b1 [256], W2 [256, 128], b2 [128], W3 [128, 1], b3 [1],
Wg [256, 256], bg [256].  Output [2, 512, 256] f32.

Sharding: core c handles batch b = c // 4 and query rows
i in [128*(c%4), 128*(c%4)+128).  Params replicated; all per-core
variation is input data (same NEFF on all 8 cores).

Per core (i-block of 128 query rows, j = all 512 nodes):
  leftb[c,i]  = (x_i @ W1[:D])[c] + b1[c]          (PE prologue)
  rightT[c,j] = (x_j @ W1[D:])[c]                  (PE prologue, bf16)
  per 2-row group g (pipelined, LAG=4):
    hT[c,j]   = relu(rightT[c,j] + leftb[c,i])     (DVE dual-scalar op, bf16,
                  4x_2p rate; one op per (row, c-tile))
    h2p[k,j]  = sum_c hT[c,j] * (TSCALE*W2*|W3|)[c,k]  (PE, 2 accum bf16
                  matmuls per row)
    t[k,j]    = relu(h2p[k,j] + TSCALE*|W3_k| b2_k)    (psum->sbuf FP8;
                  f32 psum input forces 1x rate on any engine so fp8 out
                  is free; rotates ACT(3/4)/GPSIMD(1/4) to keep DVE on hT)
    edge[2g,j], edge[2g+1,j] += sum_k sign(W3_k) t_u[k,j]
                  (ONE DoubleRow fp8 matmul per group: contraction 2x128
                  pairs both rows' t tensors; a two-plane shifted sign
                  stationary routes row u's reduce to psum partition r0+u
                  of its 32-col group; rows 0-63 / 64-127 in separate psum
                  banks so the first half's softmax runs under the second
                  half of the loop)
  edge[i,:] == TSCALE * h2(i,j,:) @ W3 up to fp8 rounding of t (|W3| and
  TSCALE folded into W2/b2, sign into the reduction weights; relu is
  positively homogeneous; 1/TSCALE folded into the tanh input scale)
  s = tanh(.5*edge + .5*b3)  => sigmoid without a table switch
  adj = .5*badj*(1+s) + eye = .5*badj*s + (.5*badj + eye)
  adjn = softmax(adj) rowwise (exp with fused accum row-sum, reciprocal)
  out = (adjn @ x_b) @ Wg + bg  (PE transposes + 2 matmul chains)

Inputs are packed host-side into 4 DRAM tensors (one bf16 critical, one
f32 critical, one fp8 sign table, one f32 late) so the prologue is 4
large DMAs instead of 26 small ones.
"""

import ml_dtypes
import numpy as np

import concourse.bacc as bacc
import concourse.bass as bass
import concourse.mybir as mybir
import concourse.tile as tile
from concourse.bass_utils import run_bass_kernel_spmd

F32 = mybir.dt.float32
BF16 = mybir.dt.bfloat16
FP8 = mybir.dt.float8e4
AF = mybir.ActivationFunctionType
OP = mybir.AluOpType
PM = mybir.MatmulPerfMode

P = 128      # partitions / i-block
N = 512      # nodes (j dim)
D = 256      # input dim
H = 256      # hidden (c dim, 2 partition tiles)
H2 = 128     # second hidden (k dim)
NCORES = 8
LAG = 4      # software-pipeline depth (in 2-row groups)

TSCALE = 1024.0  # fp8 scale for t = relu(h2p + b2s); folded into W2s/b2s

# --- packed-input layouts (free-dim element offsets) ---
# bigh (bf16): xt[2]@512, w1b[2]@256(c x2 tiles of 256? see below), w2s[2]@128,
#              wgb[2]@256
#   xt tiles:   [0:512), [512:1024)         (d-tile rows x j)
#   w1b tiles:  [1024:1280), [1280:1536)    (d-tile rows x c)  (256 each)
#   w2s tiles:  [1536:1664), [1664:1792)    (c-tile rows x k; pre-scaled by
#               TSCALE*|w3| so t lands in fp8-normal range)
#   wgb tiles:  [1792:2048), [2048:2304)    (d-tile rows x h, bf16 Wg for the
#               cheap tail projection)
BH_XT = 0
BH_W1B = 1024
BH_W2S = 1536
BH_WGB = 1792
BH_TOT = 2304
# bigp (fp8): paired-sign stationary for the DoubleRow edge reduce.
# sg2 [128, 2, 192]: plane 0 has sign(w3) at col 64, plane 1 at col 65; the
# slice [:, :, 64-r : 192-r] puts the signs at local cols (r, r+1) of a
# 128-wide stationary.  Full-width because DoubleRow is incompatible with
# PE column tiling (XBUS budget): each reduce matmul must write all 128
# psum partitions of its bank (zeros except rows 2g, 2g+1).
BP_TOT = 384
# bigf (f32, critical): xti[2]@128, w1a[2]@256, b1c@2, b2sc@1, halfb3@1
BF_XTI = 0
BF_W1A = 256
BF_B1C = 768
BF_B2SC = 770
BF_HB3 = 771
BF_TOT = 772
# bigl (f32, late): xb[4]@256, bha@512, bhae@512, ident@128, bgt@256
BL_XB = 0
BL_BHA = 1024
BL_BHAE = 1536
BL_ID = 2048
BL_BGT = 2176
BL_TOT = 2432


def _build_program(reps=1):
    """reps>1 wraps the whole kernel body in a For_i loop — used only by
    the timing bench (wall-clock regression over reps)."""
    import contextlib

    nc = bacc.Bacc("TRN2", target_bir_lowering=False, debug=False)

    bigh = nc.dram_tensor("bigh", [P, BH_TOT], BF16, kind="ExternalInput").ap()
    bigf = nc.dram_tensor("bigf", [P, BF_TOT], F32, kind="ExternalInput").ap()
    bigl = nc.dram_tensor("bigl", [P, BL_TOT], F32, kind="ExternalInput").ap()
    bigp = nc.dram_tensor("bigp", [P, BP_TOT], FP8, kind="ExternalInput").ap()
    out_d = nc.dram_tensor("out", [P, D], F32, kind="ExternalOutput").ap()

    with tile.TileContext(nc) as tc:
        with (
            tc.tile_pool(name="const", bufs=1) as const,
            tc.tile_pool(name="work", bufs=8) as work,
            tc.tile_pool(name="pedge", bufs=1, space="PSUM") as pedge,
        ):
            bh = const.tile([P, BH_TOT], BF16)
            bf = const.tile([P, BF_TOT], F32)
            bl = const.tile([P, BL_TOT], F32)
            bp = const.tile([P, 2, 192], FP8)
            loop_cm = tc.For_i(0, reps, 1) if reps > 1 else contextlib.nullcontext()
            loop_cm.__enter__()
            # two critical input DMAs on separate queue engines, late bigl after
            nc.sync.dma_start(bh[:], bigh)
            nc.gpsimd.dma_start(bf[:], bigf)
            nc.gpsimd.dma_start(bp[:], bigp)
            nc.gpsimd.dma_start(bl[:], bigl)

            def xt_sb(dt):
                return bh[:, BH_XT + 512 * dt : BH_XT + 512 * (dt + 1)]

            def w1b_sb(dt, csl):
                base = BH_W1B + 256 * dt
                return bh[:, base + csl * 128 : base + csl * 128 + 128]

            def w2s_sb(ct):
                return bh[:, BH_W2S + 128 * ct : BH_W2S + 128 * (ct + 1)]

            def wgb_sb(dt):
                return bh[:, BH_WGB + 256 * dt : BH_WGB + 256 * (dt + 1)]

            def xti_sb(dt):
                return bf[:, BF_XTI + 128 * dt : BF_XTI + 128 * (dt + 1)]

            def w1a_sb(dt, csl):
                base = BF_W1A + 256 * dt
                return bf[:, base + csl * 128 : base + csl * 128 + 128]

            b1c_sb = bf[:, BF_B1C : BF_B1C + 2]
            b2sc_sb = bf[:, BF_B2SC : BF_B2SC + 1]
            halfb3_sb = bf[:, BF_HB3 : BF_HB3 + 1]

            def xb_sb(jt):
                return bl[:, BL_XB + 256 * jt : BL_XB + 256 * (jt + 1)]

            bha_sb = bl[:, BL_BHA : BL_BHA + 512]
            bhae_sb = bl[:, BL_BHAE : BL_BHAE + 512]
            ident_sb = bl[:, BL_ID : BL_ID + 128]
            bgt_sb = bl[:, BL_BGT : BL_BGT + 256]

            # Preload the exp/tanh/relu activation table set early so the
            # ~2.7us table DMA overlaps the input DMAs.
            warm = const.tile([P, 1], F32)
            nc.vector.memset(warm[:], 0.0)
            nc.scalar.activation(warm[:], warm[:], AF.Exp)

            # edge logits split across two PSUM banks by row-half so the
            # first half's softmax chain can read bank A while the PE is
            # still accumulating rows 64..127 into bank B (no same-bank
            # PE-write/ACT-read serialization).
            edge_ps_a = pedge.tile([P, N], F32, tag="edgea")
            edge_ps_b = pedge.tile([P, N], F32, tag="edgeb")
            edge_banks = (edge_ps_a, edge_ps_b)
            scratch_sb = const.tile([P, N], BF16)
            nc.vector.memset(scratch_sb[:], 0.0)

            with tc.tile_pool(name="ph2", bufs=3, space="PSUM") as ph2:
                ptail = None
                # Warm the PE HAM clock-gate (~3.4us of dummy matmul activity
                # with no DMA dependency) while the input DMAs are in flight,
                # so the real prologue runs at 2.4 GHz instead of 1.2.
                wps = ph2.tile([P, 2, N], F32, tag="h2")
                for w in range(8):
                    nc.tensor.matmul(
                        wps[:, 0, :], scratch_sb[:, :P], scratch_sb[:],
                        start=True, stop=True,
                    )
                nc.vector.tensor_copy(warm[:], wps[:, 0, 0:1])

                # ---- prologue: rightT / leftb ----
                rightT_sb = const.tile([P, 2, N], BF16)
                leftb_sb = const.tile([P, 2, P], F32)
                for ct in range(2):
                    ps = ph2.tile([P, 2, N], F32, tag="h2")
                    for dt in range(2):
                        nc.tensor.matmul(
                            ps[:, 0, :],
                            w1b_sb(dt, ct),
                            xt_sb(dt),
                            start=(dt == 0),
                            stop=(dt == 1),
                        )
                    # one eviction per engine so they run in parallel
                    if ct == 0:
                        nc.scalar.copy(rightT_sb[:, ct, :], ps[:, 0, :])
                    else:
                        nc.vector.tensor_copy(rightT_sb[:, ct, :], ps[:, 0, :])
                for ct in range(2):
                    ps = ph2.tile([P, 2, N], F32, tag="h2")
                    for dt in range(2):
                        nc.tensor.matmul(
                            ps[:, 0, :P],
                            w1a_sb(dt, ct),
                            xti_sb(dt),
                            start=(dt == 0),
                            stop=(dt == 1),
                        )
                    if ct == 0:
                        nc.scalar.activation(
                            leftb_sb[:, ct, :], ps[:, 0, :P], AF.Identity,
                            bias=b1c_sb[:, ct : ct + 1], scale=1.0,
                        )
                    else:
                        nc.vector.tensor_scalar(
                            leftb_sb[:, ct, :], ps[:, 0, :P],
                            b1c_sb[:, ct : ct + 1], None, op0=OP.add,
                        )

                # softmax tiles written half-at-a-time (first half overlaps
                # the main loop's second half)
                s_sb = const.tile([P, N], F32)
                m1 = const.tile([P, N], F32)
                m2 = const.tile([P, N], F32)
                adjexp = const.tile([P, N], F32)
                rowsum = const.tile([P, 1], F32)
                invs = const.tile([P, 1], F32)
                adjnT0 = const.tile([P, 4, 64], F32)
                adjnT1 = const.tile([P, 4, 64], F32)
                adjnTh = (adjnT0, adjnT1)
                aggTb0 = const.tile([P, P], BF16)
                aggTb1 = const.tile([P, P], BF16)
                aggTb = (aggTb0, aggTb1)
                rsp = const.tile([P, 2], F32)
                out_ns = const.tile([P, D], F32)
                out_sb = const.tile([P, D], F32)

                def softmax_half(h, jsplit=1):
                    rs = slice(64 * h, 64 * h + 64)
                    eb = edge_banks[h]
                    # edge PSUM carries TSCALE*edge; fold 1/TSCALE into the
                    # tanh input scale (sigmoid(z) = .5 + .5*tanh(z/2)).
                    # jsplit=2 pipelines the chain in j-halves to cut its
                    # serial latency (used for the final half, where the
                    # chain is otherwise bare on the critical path).
                    w = N // jsplit
                    for q in range(jsplit):
                        js = slice(q * w, (q + 1) * w)
                        nc.scalar.activation(
                            s_sb[rs, js], eb[rs, js], AF.Tanh,
                            bias=halfb3_sb[rs, :], scale=0.5 / TSCALE,
                        )
                        nc.vector.tensor_mul(
                            m1[rs, js], s_sb[rs, js], bha_sb[rs, js]
                        )
                        nc.vector.tensor_add(
                            m2[rs, js], m1[rs, js], bhae_sb[rs, js]
                        )
                        nc.scalar.activation(
                            adjexp[rs, js], m2[rs, js], AF.Exp,
                            accum_out=rsp[rs, q : q + 1],
                        )
                    if jsplit == 1:
                        nc.vector.reciprocal(invs[rs, :], rsp[rs, 0:1])
                    else:
                        nc.vector.tensor_add(
                            rowsum[rs, :], rsp[rs, 0:1], rsp[rs, 1:2]
                        )
                        nc.vector.reciprocal(invs[rs, :], rowsum[rs, :])

                def tail_half(h, ptail):
                    # transpose/aggregate/project query rows 64h..64h+63.
                    # half 0 runs inside the main loop right after its
                    # softmax.  PSUM scratch is this half's edge bank: its
                    # reduce accumulation is closed and the softmax tanh was
                    # its only reader.  Sub-regions: transposes at cols
                    # 64*jt, aggregation at 256+64*dh, projection reuses
                    # cols 0:256 (transitively after the transposes, so no
                    # false serialization).
                    ih = slice(64 * h, 64 * h + 64)
                    scr = edge_banks[h]
                    # all transposes before any copy: region tracking on the
                    # scratch bank is tile-granular, so a copy (reader)
                    # emitted before a transpose (writer) would serialize
                    # them pairwise
                    for jt in range(4):
                        nc.tensor.transpose(
                            scr[:, 64 * jt : 64 * jt + 64],
                            adjexp[ih, bass.ts(jt, P)],
                            ident_sb[ih, 64 * h : 64 * h + 64],
                        )
                    # single merged eviction of all four transposed blocks
                    nc.vector.tensor_copy(adjnTh[h][:], scr[:, 0:256])
                    for dh in range(2):
                        pa = scr[:, 256 + 64 * dh : 320 + 64 * dh]
                        for jt in range(4):
                            nc.tensor.matmul(
                                pa,
                                xb_sb(jt)[:, bass.ts(dh, P)],
                                adjnTh[h][:, jt, :],
                                start=(jt == 0),
                                stop=(jt == 3),
                            )
                    # single merged psum->sbuf eviction of both agg halves,
                    # converting to bf16 for the cheap projection matmuls
                    nc.vector.tensor_copy(aggTb[h][:], scr[:, 256:384])
                    po = scr[ih, 0:256]
                    for dh in range(2):
                        nc.tensor.matmul(
                            po, aggTb[h][:, 64 * dh : 64 * dh + 64], wgb_sb(dh),
                            start=(dh == 0), stop=(dh == 1),
                        )
                    # deferred softmax normalization: row scale commutes to
                    # the end and is applied per output partition
                    nc.scalar.activation(
                        out_ns[ih, :], po, AF.Identity,
                        bias=0.0, scale=invs[ih, :],
                    )
                    nc.vector.tensor_add(
                        out_sb[ih, :], out_ns[ih, :], bgt_sb[ih, :]
                    )
                    nc.sync.dma_start(out_d[ih, :], out_sb[ih, :])

                # ---- main loop over the 128 query rows, 2 rows per group ----
                NG = P // 2
                h2ps = {}
                for step in range(NG + LAG):
                    if step < NG:
                        g = step
                        hts = []
                        for u in range(2):
                            i = 2 * g + u
                            ht0 = work.tile([P, N], BF16, tag=f"ht0{u}")
                            ht1 = work.tile([P, N], BF16, tag=f"ht1{u}")
                            # GPSIMD (SBUF-only) takes one of the four hT ops
                            # on odd groups to relieve DVE
                            eng1 = (
                                nc.gpsimd
                                if (g % 2 == 1 and u == 1)
                                else nc.vector
                            )
                            nc.vector.tensor_scalar(
                                ht0[:], rightT_sb[:, 0, :],
                                leftb_sb[:, 0, i : i + 1], 0.0,
                                op0=OP.add, op1=OP.max,
                            )
                            eng1.tensor_scalar(
                                ht1[:], rightT_sb[:, 1, :],
                                leftb_sb[:, 1, i : i + 1], 0.0,
                                op0=OP.add, op1=OP.max,
                            )
                            hts.append((ht0, ht1))
                        ps = ph2.tile([P, 2, N], F32, tag="h2")
                        for u in range(2):
                            nc.tensor.matmul(
                                ps[:, u, :], w2s_sb(0), hts[u][0][:],
                                start=True, stop=False,
                            )
                        for u in range(2):
                            nc.tensor.matmul(
                                ps[:, u, :], w2s_sb(1), hts[u][1][:],
                                start=False, stop=True,
                            )
                        h2ps[g] = ps
                    gj = step - LAG
                    if 0 <= gj < NG:
                        # evict h2p -> t in fp8 (psum f32 input forces 1x rate
                        # on every engine, so the fp8 output is free).  GPSIMD
                        # cannot read PSUM, so ACT takes 7/8 and DVE 1/8.
                        t_sb = work.tile([P, 2, N], FP8, tag="tt")
                        if gj >= NG - 4:
                            # end-of-loop drain: nothing overlaps these, so
                            # halve the latency by splitting rows across two
                            # engines running concurrently
                            ps = h2ps.pop(gj)
                            nc.scalar.activation(
                                t_sb[:, 0, :], ps[:, 0, :], AF.Relu,
                                bias=b2sc_sb[:], scale=1.0,
                            )
                            nc.vector.tensor_scalar(
                                t_sb[:, 1, :], ps[:, 1, :],
                                b2sc_sb[:], 0.0, op0=OP.add, op1=OP.max,
                            )
                        elif gj % 8 == 3:
                            nc.vector.tensor_scalar(
                                t_sb[:], h2ps.pop(gj)[:],
                                b2sc_sb[:], 0.0, op0=OP.add, op1=OP.max,
                            )
                        else:
                            nc.scalar.activation(
                                t_sb[:], h2ps.pop(gj)[:], AF.Relu,
                                bias=b2sc_sb[:], scale=1.0,
                            )
                        # paired edge reduce: one DoubleRow fp8 matmul
                        # accumulates rows 2gj and 2gj+1 of the edge logits
                        # (contraction 2x128, signs on planes 0/1 at shifted
                        # columns r0/r0+1 of a full-width stationary)
                        r0 = (2 * gj) % 64
                        nc.tensor.matmul(
                            edge_banks[gj // 32][:, :],
                            bp[:, :, 64 - r0 : 192 - r0],
                            t_sb[:],
                            start=(gj % 32 == 0),
                            stop=(gj % 32 == 31),
                            perf_mode=PM.DoubleRow,
                        )
                        if 2 * gj + 1 == 63:
                            softmax_half(0)
                            tail_half(0, ptail)

                # ---- tail: second-half softmax, aggregate, project ----
                softmax_half(1, jsplit=2)
                tail_half(1, ptail)
            loop_cm.__exit__(None, None, None)

    nc.compile()
    return nc


_NC = None


def _get_program():
    global _NC
    if _NC is None:
        _NC = _build_program()
    return _NC


def _core_inputs(x, base_adj, W1, b1, W2, b2, W3, b3, Wg, bg, core):
    b, blk = divmod(core, 4)
    i0 = blk * P
    f32 = np.float32
    bf16 = ml_dtypes.bfloat16

    xbf = np.ascontiguousarray(x[b], dtype=f32)               # [512, 256]
    xtf = np.ascontiguousarray(xbf.T)                         # [256, 512]
    w3 = np.asarray(W3, dtype=f32)[:, 0]                      # [128]

    sg2 = np.zeros((P, 2, 192), dtype=f32)
    sg2[:, 0, 64] = np.sign(w3)
    sg2[:, 1, 65] = np.sign(w3)
    w2s = np.ascontiguousarray(
        W2.astype(f32) * (np.abs(w3) * TSCALE)[None, :]
    )

    bha = 0.5 * base_adj[b, i0 : i0 + P, :].astype(f32)
    eye = np.zeros((P, N), dtype=f32)
    eye[np.arange(P), i0 + np.arange(P)] = 1.0

    W1 = np.asarray(W1, f32)

    Wgf = np.asarray(Wg, f32)
    bigh = np.concatenate(
        [
            xtf[:128, :], xtf[128:, :],                        # xt d-tiles
            W1[D:D + 128, :], W1[D + 128 :, :],                # w1b d-tiles
            w2s[:128, :], w2s[128:, :],                        # w2s c-tiles
            Wgf[:128, :], Wgf[128:, :],                        # wgb d-tiles
        ],
        axis=1,
    ).astype(bf16)
    xtif = np.ascontiguousarray(xbf[i0 : i0 + P, :].T)         # [256, 128]
    bigf = np.concatenate(
        [
            xtif[:128, :], xtif[128:, :],                      # xti d-tiles
            W1[:128, :], W1[128:D, :],                         # w1a d-tiles
            np.asarray(b1, f32).reshape(2, P).T,               # b1c [P, 2]
            (TSCALE * np.abs(w3) * np.asarray(b2, f32)).reshape(P, 1),  # b2sc
            np.full((P, 1), 0.5 * float(np.asarray(b3).reshape(-1)[0]), f32),
        ],
        axis=1,
    )
    bigl = np.concatenate(
        [
            xbf[0:128], xbf[128:256], xbf[256:384], xbf[384:],  # xb j-tiles
            bha, bha + eye,
            np.eye(P, dtype=f32),
            np.tile(np.asarray(bg, f32)[None, :], (P, 1)),
        ],
        axis=1,
    )
    assert bigh.shape[1] == BH_TOT and bigf.shape[1] == BF_TOT
    assert bigl.shape[1] == BL_TOT
    return {
        "bigh": np.ascontiguousarray(bigh),
        "bigf": np.ascontiguousarray(bigf),
        "bigl": np.ascontiguousarray(bigl),
        "bigp": np.ascontiguousarray(
            sg2.reshape(P, BP_TOT).astype(ml_dtypes.float8_e4m3)
        ),
    }


def run(trace=False, **inputs):
    nc = _get_program()
    inputs = {k: np.asarray(v) for k, v in inputs.items()}
    in_maps = [_core_inputs(core=c, **inputs) for c in range(NCORES)]
    res = run_bass_kernel_spmd(
        nc, in_maps, core_ids=list(range(NCORES)), trace=trace
    )
    out = np.empty((2, N, D), dtype=np.float32)
    for c in range(NCORES):
        b, blk = divmod(c, 4)
        out[b, blk * P : (blk + 1) * P, :] = res.results[c]["out"]
    return out, res


def kernel(**inputs):
    out, _ = run(**inputs)
    return out



# revision 51
# speedup vs baseline: 3.0077x; 3.0077x over previous
"""DynamicGCN edge-MLP message passing kernel for 8x trn2 NeuronCores.

Shapes (hardcoded): x [2, 512, 256] f32, base_adj [2, 512, 512] f32,
W1 [512, 256], b1 [256], W2 [256, 128], b2 [128], W3 [128, 1], b3 [1],
Wg [256, 256], bg [256].  Output [2, 512, 256] f32.

Sharding: core c handles batch b = c // 4 and query rows
i in [128*(c%4), 128*(c%4)+128).  Params replicated; all per-core
variation is input data (same NEFF on all 8 cores).

Per core (i-block of 128 query rows, j = all 512 nodes):
  leftb[c,i]  = (x_i @ W1[:D])[c] + b1[c]          (PE prologue)
  rightT[c,j] = (x_j @ W1[D:])[c]                  (PE prologue, bf16)
  per 2-row group g (pipelined, LAG=4):
    hT[c,j]   = relu(rightT[c,j] + leftb[c,i])     (DVE dual-scalar op, bf16,
                  4x_2p rate; one op per (row, c-tile))
    h2p[k,j]  = sum_c hT[c,j] * (TSCALE*W2*|W3|)[c,k]  (PE, 2 accum bf16
                  matmuls per row)
    t[k,j]    = relu(h2p[k,j] + TSCALE*|W3_k| b2_k)    (psum->sbuf FP8;
                  f32 psum input forces 1x rate on any engine so fp8 out
                  is free; rotates ACT(3/4)/GPSIMD(1/4) to keep DVE on hT)
    edge[2g,j], edge[2g+1,j] += sum_k sign(W3_k) t_u[k,j]
                  (ONE DoubleRow fp8 matmul per group: contraction 2x128
                  pairs both rows' t tensors; a two-plane shifted sign
                  stationary routes row u's reduce to psum partition r0+u
                  of its 32-col group; rows 0-63 / 64-127 in separate psum
                  banks so the first half's softmax runs under the second
                  half of the loop)
  edge[i,:] == TSCALE * h2(i,j,:) @ W3 up to fp8 rounding of t (|W3| and
  TSCALE folded into W2/b2, sign into the reduction weights; relu is
  positively homogeneous; 1/TSCALE folded into the tanh input scale)
  s = tanh(.5*edge + .5*b3)  => sigmoid without a table switch
  adj = .5*badj*(1+s) + eye = .5*badj*s + (.5*badj + eye)
  adjn = softmax(adj) rowwise (exp with fused accum row-sum, reciprocal)
  out = (adjn @ x_b) @ Wg + bg  (PE transposes + 2 matmul chains)

Inputs are packed host-side into 4 DRAM tensors (one bf16 critical, one
f32 critical, one fp8 sign table, one f32 late) so the prologue is 4
large DMAs instead of 26 small ones.
"""

import ml_dtypes
import numpy as np

import concourse.bacc as bacc
import concourse.bass as bass
import concourse.mybir as mybir
import concourse.tile as tile
from concourse.bass_utils import run_bass_kernel_spmd

F32 = mybir.dt.float32
BF16 = mybir.dt.bfloat16
FP8 = mybir.dt.float8e4
AF = mybir.ActivationFunctionType
OP = mybir.AluOpType
PM = mybir.MatmulPerfMode

P = 128      # partitions / i-block
N = 512      # nodes (j dim)
D = 256      # input dim
H = 256      # hidden (c dim, 2 partition tiles)
H2 = 128     # second hidden (k dim)
NCORES = 8
LAG = 3      # software-pipeline depth (in 2-row groups)

TSCALE = 1024.0  # fp8 scale for t = relu(h2p + b2s); folded into W2s/b2s

# --- packed-input layouts (free-dim element offsets) ---
# bigh (bf16): xt[2]@512, w1b[2]@256(c x2 tiles of 256? see below), w2s[2]@128,
#              wgb[2]@256, xb[4]@256
#   xt tiles:   [0:512), [512:1024)         (d-tile rows x j)
#   w1b tiles:  [1024:1280), [1280:1536)    (d-tile rows x c)  (256 each)
#   w2s tiles:  [1536:1664), [1664:1792)    (c-tile rows x k; pre-scaled by
#               TSCALE*|w3| so t lands in fp8-normal range)
#   wgb tiles:  [1792:2048), [2048:2304)    (d-tile rows x h, bf16 Wg for the
#               cheap tail projection)
#   xb tiles:   [2304:3328)                 (j-tile rows x d, bf16 x for the
#               tail aggregation stationary)
BH_XT = 0
BH_W1B = 1024
BH_W2S = 1536
BH_WGB = 1792
BH_XB = 2304
BH_TOT = 3328
# bigp (fp8): paired-sign stationary for the DoubleRow edge reduce.
# sg2 [128, 2, 256]: plane 0 has sign(w3) at col 128, plane 1 at col 129;
# the slice [:, :, 128-r : 256-r] puts the signs at local cols (r, r+1) of
# a 128-wide stationary, r = global row index 2g (psum partition within the
# bank).  Full-width because DoubleRow is incompatible with PE column
# tiling (XBUS budget): each reduce matmul writes all 128 psum partitions
# of its bank (zeros except rows 2g, 2g+1).
BP_TOT = 512
# bigf (f32, critical): xti[2]@128, w1a[2]@256, b1c@2, b2sc@1, halfb3@1
BF_XTI = 0
BF_W1A = 256
BF_B1C = 768
BF_B2SC = 770
BF_HB3 = 771
BF_TOT = 772
# bigl (f32, late): bha@512, bhae@512, ident@128, bgt@256
BL_BHA = 0
BL_BHAE = 512
BL_ID = 1024
BL_BGT = 1152
BL_TOT = 1408


def _build_program(reps=1, use_dr=True, use_pool=False, t_fp8=True):
    """reps>1 wraps the whole kernel body in a For_i loop — used only by
    the timing bench (wall-clock regression over reps).  The use_* flags
    are dev-only ablations for hardware A/B timing (default = shipping
    configuration)."""
    import contextlib

    nc = bacc.Bacc("TRN2", target_bir_lowering=False, debug=False)

    bigh = nc.dram_tensor("bigh", [P, BH_TOT], BF16, kind="ExternalInput").ap()
    bigf = nc.dram_tensor("bigf", [P, BF_TOT], F32, kind="ExternalInput").ap()
    bigl = nc.dram_tensor("bigl", [P, BL_TOT], F32, kind="ExternalInput").ap()
    bigp = nc.dram_tensor("bigp", [P, BP_TOT], FP8, kind="ExternalInput").ap()
    out_d = nc.dram_tensor("out", [P, D], F32, kind="ExternalOutput").ap()

    with tile.TileContext(nc) as tc:
        with (
            tc.tile_pool(name="const", bufs=1) as const,
            tc.tile_pool(name="work", bufs=8) as work,
            tc.tile_pool(name="pedge", bufs=1, space="PSUM") as pedge,
        ):
            bh = const.tile([P, BH_TOT], BF16)
            bf = const.tile([P, BF_TOT], F32)
            bl = const.tile([P, BL_TOT], F32)
            bp = const.tile([P, 2, 256], FP8)
            loop_cm = tc.For_i(0, reps, 1) if reps > 1 else contextlib.nullcontext()
            loop_cm.__enter__()
            # input DMAs fan out across four queue engines so their
            # descriptor generation runs in parallel (the issuing engines
            # are all idle at kernel start)
            nc.sync.dma_start(bh[:], bigh)
            nc.gpsimd.dma_start(bf[:], bigf)
            nc.scalar.dma_start(bp[:], bigp)
            nc.gpsimd.dma_start(bl[:], bigl)

            def xt_sb(dt):
                return bh[:, BH_XT + 512 * dt : BH_XT + 512 * (dt + 1)]

            def w1b_sb(dt, csl):
                base = BH_W1B + 256 * dt
                return bh[:, base + csl * 128 : base + csl * 128 + 128]

            def w2s_sb(ct):
                return bh[:, BH_W2S + 128 * ct : BH_W2S + 128 * (ct + 1)]

            def wgb_sb(dt):
                return bh[:, BH_WGB + 256 * dt : BH_WGB + 256 * (dt + 1)]

            def xti_sb(dt):
                return bf[:, BF_XTI + 128 * dt : BF_XTI + 128 * (dt + 1)]

            def w1a_sb(dt, csl):
                base = BF_W1A + 256 * dt
                return bf[:, base + csl * 128 : base + csl * 128 + 128]

            b1c_sb = bf[:, BF_B1C : BF_B1C + 2]
            b2sc_sb = bf[:, BF_B2SC : BF_B2SC + 1]
            halfb3_sb = bf[:, BF_HB3 : BF_HB3 + 1]

            def xb_sb(jt):
                return bh[:, BH_XB + 256 * jt : BH_XB + 256 * (jt + 1)]

            bha_sb = bl[:, BL_BHA : BL_BHA + 512]
            bhae_sb = bl[:, BL_BHAE : BL_BHAE + 512]
            ident_sb = bl[:, BL_ID : BL_ID + 128]
            bgt_sb = bl[:, BL_BGT : BL_BGT + 256]

            # Preload the exp/tanh/relu activation table set early so the
            # ~2.7us table DMA overlaps the input DMAs.
            warm = const.tile([P, 1], F32)
            nc.vector.memset(warm[:], 0.0)
            nc.scalar.activation(warm[:], warm[:], AF.Exp)

            # edge logits split across two PSUM banks by row-half so the
            # first half's softmax chain can read bank A while the PE is
            # still accumulating rows 64..127 into bank B (no same-bank
            # PE-write/ACT-read serialization).
            edge_ps_a = pedge.tile([P, N], F32, tag="edgea")
            edge_ps_b = pedge.tile([P, N], F32, tag="edgeb")
            edge_banks = (edge_ps_a, edge_ps_b)
            scratch_sb = const.tile([P, N], BF16)
            nc.vector.memset(scratch_sb[:], 0.0)

            with tc.tile_pool(name="ph2", bufs=3, space="PSUM") as ph2:
                ptail = None
                # Warm the PE HAM clock-gate (~3.4us of dummy matmul activity
                # with no DMA dependency) while the input DMAs are in flight,
                # so the real prologue runs at 2.4 GHz instead of 1.2.
                wps = ph2.tile([P, 2, N], F32, tag="h2")
                for w in range(8):
                    nc.tensor.matmul(
                        wps[:, 0, :], scratch_sb[:, :P], scratch_sb[:],
                        start=True, stop=True,
                    )
                nc.vector.tensor_copy(warm[:], wps[:, 0, 0:1])

                # ---- prologue: rightT / leftb ----
                rightT_sb = const.tile([P, 2, N], BF16)
                leftb_sb = const.tile([P, 2, P], F32)
                for ct in range(2):
                    ps = ph2.tile([P, 2, N], F32, tag="h2")
                    for dt in range(2):
                        nc.tensor.matmul(
                            ps[:, 0, :],
                            w1b_sb(dt, ct),
                            xt_sb(dt),
                            start=(dt == 0),
                            stop=(dt == 1),
                        )
                    # one eviction per engine so they run in parallel
                    if ct == 0:
                        nc.scalar.copy(rightT_sb[:, ct, :], ps[:, 0, :])
                    else:
                        nc.vector.tensor_copy(rightT_sb[:, ct, :], ps[:, 0, :])
                for ct in range(2):
                    ps = ph2.tile([P, 2, N], F32, tag="h2")
                    for dt in range(2):
                        nc.tensor.matmul(
                            ps[:, 0, :P],
                            w1a_sb(dt, ct),
                            xti_sb(dt),
                            start=(dt == 0),
                            stop=(dt == 1),
                        )
                    if ct == 0:
                        nc.scalar.activation(
                            leftb_sb[:, ct, :], ps[:, 0, :P], AF.Identity,
                            bias=b1c_sb[:, ct : ct + 1], scale=1.0,
                        )
                    else:
                        nc.vector.tensor_scalar(
                            leftb_sb[:, ct, :], ps[:, 0, :P],
                            b1c_sb[:, ct : ct + 1], None, op0=OP.add,
                        )

                # softmax tiles written half-at-a-time (first half overlaps
                # the main loop's second half)
                s_sb = const.tile([P, N], F32)
                m1 = const.tile([P, N], F32)
                m2 = const.tile([P, N], F32)
                adjexp = const.tile([P, N], F32)
                rowsum = const.tile([P, 1], F32)
                invs = const.tile([P, 1], F32)
                adjnT0 = const.tile([P, 4, 64], BF16)
                adjnT1 = const.tile([P, 4, 64], BF16)
                adjnTh = (adjnT0, adjnT1)
                aggTb0 = const.tile([P, P], BF16)
                aggTb1 = const.tile([P, P], BF16)
                aggTb = (aggTb0, aggTb1)
                rsp = const.tile([P, 2], F32)
                out_ns = const.tile([P, D], F32)
                out_sb = const.tile([P, D], F32)

                def softmax_half(h, jsplit=1):
                    rs = slice(64 * h, 64 * h + 64)
                    eb = edge_banks[h]
                    # edge PSUM carries TSCALE*edge; fold 1/TSCALE into the
                    # tanh input scale (sigmoid(z) = .5 + .5*tanh(z/2)).
                    # jsplit=2 pipelines the chain in j-halves to cut its
                    # serial latency (used for the final half, where the
                    # chain is otherwise bare on the critical path).
                    w = N // jsplit
                    for q in range(jsplit):
                        js = slice(q * w, (q + 1) * w)
                        nc.scalar.activation(
                            s_sb[rs, js], eb[rs, js], AF.Tanh,
                            bias=halfb3_sb[rs, :], scale=0.5 / TSCALE,
                        )
                        nc.vector.tensor_mul(
                            m1[rs, js], s_sb[rs, js], bha_sb[rs, js]
                        )
                        nc.vector.tensor_add(
                            m2[rs, js], m1[rs, js], bhae_sb[rs, js]
                        )
                        nc.scalar.activation(
                            adjexp[rs, js], m2[rs, js], AF.Exp,
                            accum_out=rsp[rs, q : q + 1],
                        )
                    if jsplit == 1:
                        nc.vector.reciprocal(invs[rs, :], rsp[rs, 0:1])
                    else:
                        nc.vector.tensor_add(
                            rowsum[rs, :], rsp[rs, 0:1], rsp[rs, 1:2]
                        )
                        nc.vector.reciprocal(invs[rs, :], rowsum[rs, :])

                def tail_half(h, ptail):
                    # transpose/aggregate/project query rows 64h..64h+63.
                    # half 0 runs inside the main loop right after its
                    # softmax.  PSUM scratch is this half's edge bank: its
                    # reduce accumulation is closed and the softmax tanh was
                    # its only reader.  Sub-regions: transposes at cols
                    # 64*jt, aggregation at 256+64*dh, projection reuses
                    # cols 0:256 (transitively after the transposes, so no
                    # false serialization).
                    ih = slice(64 * h, 64 * h + 64)
                    scr = edge_banks[h]
                    # all transposes before any copy: region tracking on the
                    # scratch bank is tile-granular, so a copy (reader)
                    # emitted before a transpose (writer) would serialize
                    # them pairwise
                    for jt in range(4):
                        nc.tensor.transpose(
                            scr[:, 64 * jt : 64 * jt + 64],
                            adjexp[ih, bass.ts(jt, P)],
                            ident_sb[ih, 64 * h : 64 * h + 64],
                        )
                    # single merged eviction of all four transposed blocks
                    nc.vector.tensor_copy(adjnTh[h][:], scr[:, 0:256])
                    for dh in range(2):
                        pa = scr[:, 256 + 64 * dh : 320 + 64 * dh]
                        for jt in range(4):
                            nc.tensor.matmul(
                                pa,
                                xb_sb(jt)[:, bass.ts(dh, P)],
                                adjnTh[h][:, jt, :],
                                start=(jt == 0),
                                stop=(jt == 3),
                            )
                    # single merged psum->sbuf eviction of both agg halves,
                    # converting to bf16 for the cheap projection matmuls
                    nc.vector.tensor_copy(aggTb[h][:], scr[:, 256:384])
                    po = scr[ih, 0:256]
                    for dh in range(2):
                        nc.tensor.matmul(
                            po, aggTb[h][:, 64 * dh : 64 * dh + 64], wgb_sb(dh),
                            start=(dh == 0), stop=(dh == 1),
                        )
                    # deferred softmax normalization: row scale commutes to
                    # the end and is applied per output partition; both final
                    # ops on DVE so they are queue-adjacent (no cross-engine
                    # semaphore hop)
                    nc.vector.tensor_scalar(
                        out_ns[ih, :], po, invs[ih, :], None, op0=OP.mult,
                    )
                    nc.vector.tensor_add(
                        out_sb[ih, :], out_ns[ih, :], bgt_sb[ih, :]
                    )
                    nc.sync.dma_start(out_d[ih, :], out_sb[ih, :])

                # ---- main loop over the 128 query rows, 2 rows per group ----
                NG = P // 2
                h2ps = {}
                for step in range(NG + LAG):
                    if step < NG:
                        g = step
                        hts = []
                        for u in range(2):
                            i = 2 * g + u
                            ht0 = work.tile([P, N], BF16, tag=f"ht0{u}")
                            ht1 = work.tile([P, N], BF16, tag=f"ht1{u}")
                            # GPSIMD (SBUF-only) takes one of the four hT ops
                            # on odd groups to relieve DVE
                            eng1 = (
                                nc.gpsimd
                                if (use_pool and g % 2 == 1 and u == 1)
                                else nc.vector
                            )
                            nc.vector.tensor_scalar(
                                ht0[:], rightT_sb[:, 0, :],
                                leftb_sb[:, 0, i : i + 1], 0.0,
                                op0=OP.add, op1=OP.max,
                            )
                            eng1.tensor_scalar(
                                ht1[:], rightT_sb[:, 1, :],
                                leftb_sb[:, 1, i : i + 1], 0.0,
                                op0=OP.add, op1=OP.max,
                            )
                            hts.append((ht0, ht1))
                        ps = ph2.tile([P, 2, N], F32, tag="h2")
                        for u in range(2):
                            nc.tensor.matmul(
                                ps[:, u, :], w2s_sb(0), hts[u][0][:],
                                start=True, stop=False,
                            )
                        for u in range(2):
                            nc.tensor.matmul(
                                ps[:, u, :], w2s_sb(1), hts[u][1][:],
                                start=False, stop=True,
                            )
                        h2ps[g] = ps
                    gj = step - LAG
                    if 0 <= gj < NG:
                        # evict h2p -> t in fp8 (psum f32 input forces 1x rate
                        # on every engine, so the fp8 output is free).  GPSIMD
                        # cannot read PSUM, so ACT takes 7/8 and DVE 1/8.
                        t_sb = work.tile([P, 2, N], FP8 if t_fp8 else BF16, tag="tt")
                        if gj >= NG - 4:
                            # end-of-loop drain: nothing overlaps these, so
                            # halve the latency by splitting rows across two
                            # engines running concurrently
                            ps = h2ps.pop(gj)
                            nc.scalar.activation(
                                t_sb[:, 0, :], ps[:, 0, :], AF.Relu,
                                bias=b2sc_sb[:], scale=1.0,
                            )
                            nc.vector.tensor_scalar(
                                t_sb[:, 1, :], ps[:, 1, :],
                                b2sc_sb[:], 0.0, op0=OP.add, op1=OP.max,
                            )
                        elif gj % 8 == 3:
                            nc.vector.tensor_scalar(
                                t_sb[:], h2ps.pop(gj)[:],
                                b2sc_sb[:], 0.0, op0=OP.add, op1=OP.max,
                            )
                        else:
                            nc.scalar.activation(
                                t_sb[:], h2ps.pop(gj)[:], AF.Relu,
                                bias=b2sc_sb[:], scale=1.0,
                            )
                        # paired edge reduce: one DoubleRow fp8 matmul
                        # accumulates rows 2gj and 2gj+1 of the edge logits
                        # (contraction 2x128, signs on planes 0/1 at shifted
                        # columns r0/r0+1 of a full-width stationary)
                        if use_dr:
                            r0 = 2 * gj
                            nc.tensor.matmul(
                                edge_banks[gj // 32][:, :],
                                bp[:, :, 128 - r0 : 256 - r0],
                                t_sb[:],
                                start=(gj % 32 == 0),
                                stop=(gj % 32 == 31),
                                perf_mode=PM.DoubleRow,
                            )
                        else:
                            for u in range(2):
                                r0 = 2 * gj + u
                                nc.tensor.matmul(
                                    edge_banks[gj // 32][:, :],
                                    bp[:, 0, 128 - r0 : 256 - r0],
                                    t_sb[:, u, :],
                                    start=(gj % 32 == 0 and u == 0),
                                    stop=(gj % 32 == 31 and u == 1),
                                )
                        if 2 * gj + 1 == 63:
                            softmax_half(0)
                            tail_half(0, ptail)

                # ---- tail: second-half softmax, aggregate, project ----
                softmax_half(1, jsplit=2)
                tail_half(1, ptail)
            loop_cm.__exit__(None, None, None)

    nc.compile()
    return nc


_NC = None


def _get_program():
    global _NC
    if _NC is None:
        _NC = _build_program()
    return _NC


def _core_inputs(x, base_adj, W1, b1, W2, b2, W3, b3, Wg, bg, core):
    b, blk = divmod(core, 4)
    i0 = blk * P
    f32 = np.float32
    bf16 = ml_dtypes.bfloat16

    xbf = np.ascontiguousarray(x[b], dtype=f32)               # [512, 256]
    xtf = np.ascontiguousarray(xbf.T)                         # [256, 512]
    w3 = np.asarray(W3, dtype=f32)[:, 0]                      # [128]

    sg2 = np.zeros((P, 2, 256), dtype=f32)
    sg2[:, 0, 128] = np.sign(w3)
    sg2[:, 1, 129] = np.sign(w3)
    w2s = np.ascontiguousarray(
        W2.astype(f32) * (np.abs(w3) * TSCALE)[None, :]
    )

    bha = 0.5 * base_adj[b, i0 : i0 + P, :].astype(f32)
    eye = np.zeros((P, N), dtype=f32)
    eye[np.arange(P), i0 + np.arange(P)] = 1.0

    W1 = np.asarray(W1, f32)

    Wgf = np.asarray(Wg, f32)
    bigh = np.concatenate(
        [
            xtf[:128, :], xtf[128:, :],                        # xt d-tiles
            W1[D:D + 128, :], W1[D + 128 :, :],                # w1b d-tiles
            w2s[:128, :], w2s[128:, :],                        # w2s c-tiles
            Wgf[:128, :], Wgf[128:, :],                        # wgb d-tiles
            xbf[0:128], xbf[128:256], xbf[256:384], xbf[384:],  # xb j-tiles
        ],
        axis=1,
    ).astype(bf16)
    xtif = np.ascontiguousarray(xbf[i0 : i0 + P, :].T)         # [256, 128]
    bigf = np.concatenate(
        [
            xtif[:128, :], xtif[128:, :],                      # xti d-tiles
            W1[:128, :], W1[128:D, :],                         # w1a d-tiles
            np.asarray(b1, f32).reshape(2, P).T,               # b1c [P, 2]
            (TSCALE * np.abs(w3) * np.asarray(b2, f32)).reshape(P, 1),  # b2sc
            np.full((P, 1), 0.5 * float(np.asarray(b3).reshape(-1)[0]), f32),
        ],
        axis=1,
    )
    bigl = np.concatenate(
        [
            bha, bha + eye,
            np.eye(P, dtype=f32),
            np.tile(np.asarray(bg, f32)[None, :], (P, 1)),
        ],
        axis=1,
    )
    assert bigh.shape[1] == BH_TOT and bigf.shape[1] == BF_TOT
    assert bigl.shape[1] == BL_TOT
    return {
        "bigh": np.ascontiguousarray(bigh),
        "bigf": np.ascontiguousarray(bigf),
        "bigl": np.ascontiguousarray(bigl),
        "bigp": np.ascontiguousarray(
            sg2.reshape(P, BP_TOT).astype(ml_dtypes.float8_e4m3)
        ),
    }


def run(trace=False, **inputs):
    nc = _get_program()
    inputs = {k: np.asarray(v) for k, v in inputs.items()}
    in_maps = [_core_inputs(core=c, **inputs) for c in range(NCORES)]
    res = run_bass_kernel_spmd(
        nc, in_maps, core_ids=list(range(NCORES)), trace=trace
    )
    out = np.empty((2, N, D), dtype=np.float32)
    for c in range(NCORES):
        b, blk = divmod(c, 4)
        out[b, blk * P : (blk + 1) * P, :] = res.results[c]["out"]
    return out, res


def kernel(**inputs):
    out, _ = run(**inputs)
    return out

